# revision 69
# baseline (speedup 1.0000x reference)
"""Capsule-routing kernel for 8 Trainium2 NeuronCores.

Problem: u_hat = einsum('nidk,bik->bnid', W, x); 3 rounds of dynamic
routing (softmax over n, weighted sum over i, squash, agreement update).

Sharding: input-capsule axis i (2048) split 8 ways -> 256 i per core.
Softmax over n is local; the per-iteration weighted sum s[b,n,d] is a
partial over local i, combined with an on-device AllReduce (iterations
1,2) or on the host (final iteration).

Per-core schedule (B=32, N=64, IL=256, D=32, K=16):
  sweep 1: u_hat via TensorE (block-diag x lhsT, K=64, M=128 -> psum
           [(j,b), (d,n)]), drain-cast to fp16 split between ACT and
           DVE, store to DRAM; S0 accumulated on PE via an accumulating
           ones-matmul (no DVE subaccs). AllReduce S0 (f16 payload),
           squash -> out0 [128, 2048] f16 (partition-replicated x4).
  sweep 2/3 (per pair of 4-i groups): load u16 [128,2,2048], DVE
           tmp=u16*out_rep (one 2x op, broadcast middle dim),
           halving-tree over d -> agreement into a [128,NP,2,N] f32
           mega-state; per 4-pair window: batched max-reduce, stt
           max-subtract+INV_LOG2 scale, one ACT exp, batched Z-reduce,
           reciprocal, e_norm = e*(1/Z); sm = u16*e_norm_rep in ONE 2x
           DVE op (broadcast over d); fold with constant block-ones
           lhsT accumulating s partial in psum (PE). AllReduce+squash
           between sweeps; final partial summed+squashed on host.

Layouts: u16 partition p = 32*j + b (j = i mod 4 within group), free
(d,n) d-major so d-reductions and e/out broadcasts keep the packed
last dim (n) required for DVE 2x mode.
"""
import sys
import types

sys.path.insert(0, "/opt/trn_rl_repo")

import numpy as np

from concourse import bacc, tile, mybir
from concourse.bass_utils import run_bass_kernel_spmd

f32 = mybir.dt.float32
f16 = mybir.dt.float16
AX = mybir.AxisListType
OP = mybir.AluOpType
AF = mybir.ActivationFunctionType

B, N, I, D, K = 32, 64, 2048, 32, 16
NCORES = 8
IL = I // NCORES          # 256 local input capsules
G = IL // 4               # 64 groups of 4 i
NP = G // 2               # 32 group-pairs
DN = D * N                # 2048 free elements per group, d-major
INV_LOG2 = float(1.0 / np.log(2.0))
WIN = 4                   # group-pairs per softmax batch window


def _install_ntff_hook():
    if "antenv.axon_hooks" in sys.modules:
        return
    try:
        mod = types.ModuleType("antenv.axon_hooks")
        state = {"hook": None}
        mod.set_axon_ntff_profile_hook = lambda h: state.__setitem__("hook", h)
        mod.get_axon_ntff_profile_hook = lambda: state["hook"]
        sys.modules["antenv.axon_hooks"] = mod
        import antenv
        antenv.axon_hooks = mod
        from trn_agent_boot.trn_boot import _ntff_profile_via_ctypes
        mod.set_axon_ntff_profile_hook(
            _ntff_profile_via_ctypes("/opt/axon/libaxon_pjrt.so"))
    except Exception:
        pass


def _build():
    nc = bacc.Bacc("TRN2", target_bir_lowering=False, debug=False,
                   num_devices=NCORES)

    w_t2 = nc.dram_tensor("w_t2", [NP, 128, DN], f16, kind="ExternalInput")
    x_bd = nc.dram_tensor("x_bd", [128, NP, 128], f16, kind="ExternalInput")
    s2_part = nc.dram_tensor("s2_part", [B, DN], f32, kind="ExternalOutput")

    u_store = nc.dram_tensor("u_store", [G, 128, DN], f16)
    cc_in = [nc.dram_tensor(f"cc_in{r}", [B, DN], f16) for r in range(2)]
    cc_out = [nc.dram_tensor(f"cc_out{r}", [B, DN], f16, addr_space="Shared")
              for r in range(2)]
    # tiny warm-up collective: absorbs ncfw first-call staging while
    # sweep 1 computes (collectives run on TOPSP, not the 5 engines)
    cc_win = nc.dram_tensor("cc_win", [32, 16], f16)
    cc_wout = nc.dram_tensor("cc_wout", [32, 16], f16, addr_space="Shared")

    ones4_np = np.zeros((128, 32), np.float16)
    for p in range(128):
        ones4_np[p, p % 32] = 1.0
    ones4 = nc.inline_tensor(ones4_np, name="ones4")

    core_ids = list(range(NCORES))

    with tile.TileContext(nc) as tc:
        from contextlib import ExitStack
        _late = ExitStack()
        with tc.tile_pool(name="const", bufs=1) as constp, \
             tc.tile_pool(name="tail", bufs=1) as tail, \
             tc.tile_pool(name="small", bufs=4) as small, \
             tc.tile_pool(name="bstate", bufs=1) as bstate:

            ones_sb = constp.tile([128, 32], f16)
            nc.sync.dma_start(ones_sb[:], ones4[:])
            out_rep = [constp.tile([128, DN], f16, tag=f"orep{r}",
                                   name=f"orep{r}") for r in range(2)]

            # warm-up collectives, overlapped with sweep-1 startup: each
            # ncfw collective runs faster than the previous, so two dummies
            # make both real AllReduces hit the warmed path
            wtile = constp.tile([32, 16], f16, tag="warm")
            nc.vector.memset(wtile[:], 1.0)
            nc.sync.dma_start(cc_win[:], wtile[:])
            for _ in range(2):
                nc.gpsimd.collective_compute(
                    "AllReduce", OP.add, ins=[cc_win[:]],
                    outs=[cc_wout[:]], replica_groups=[core_ids])

            def squash_to_outrep(s_sb, orep, pre_scale):
                """orep [128, (d,n)] f16 <- x4-replicated squash(s_sb*pre_scale).
                s_sb is a [B, DN] f16 AP."""
                ps2 = float(pre_scale * pre_scale)
                s3 = s_sb.rearrange("p (d n) -> p d n", n=N)
                sq = tail.tile([32, D, N], f32, tag="t_sq")
                nc.vector.tensor_mul(sq[:], s3, s3)
                cur, d = sq, D
                while d > 2:
                    nxt = tail.tile([32, d // 2, N], f32, tag=f"t_tr{d}")
                    nc.vector.tensor_add(nxt[:], cur[:, 0:d // 2, :],
                                         cur[:, d // 2:d, :])
                    cur, d = nxt, d // 2
                sn = tail.tile([32, 1, N], f32, tag="t_sn")
                nc.vector.tensor_add(sn[:], cur[:, 0:1, :], cur[:, 1:2, :])
                r_ = tail.tile([32, N], f32, tag="t_r")
                nc.scalar.activation(r_[:], sn[:, 0, :], AF.Sqrt,
                                     bias=0.0, scale=ps2)
                den = tail.tile([32, N], f32, tag="t_den")
                nc.vector.tensor_scalar(den[:], sn[:, 0, :], ps2, 1.0,
                                        OP.mult, OP.add)
                rd = tail.tile([32, N], f32, tag="t_rd")
                nc.vector.reciprocal(rd[:], den[:])
                fac = tail.tile([32, N], f16, tag="t_fac")
                nc.vector.scalar_tensor_tensor(fac[:], r_[:],
                                               float(pre_scale), rd[:],
                                               op0=OP.mult, op1=OP.mult)
                o16 = tail.tile([32, D, N], f16, tag="t_o16")
                nc.vector.tensor_mul(
                    o16[:], s3,
                    fac[:].unsqueeze(1).broadcast_to([32, D, N]))
                for j in range(4):
                    nc.sync.dma_start(
                        orep[32 * j:32 * j + 32, :],
                        o16[:].rearrange("p d n -> p (d n)"))

            def exchange_and_squash(s_sb16, r, orep, pre_scale):
                """AllReduce the [B, DN] f16 partial, squash into orep."""
                nc.sync.dma_start(cc_in[r][:], s_sb16[:])
                nc.gpsimd.collective_compute(
                    "AllReduce", OP.add, ins=[cc_in[r][:]],
                    outs=[cc_out[r][:]], replica_groups=[core_ids])
                s_all = tail.tile([B, DN], f16, tag="t_all")
                nc.sync.dma_start(s_all[:], cc_out[r][:])
                squash_to_outrep(s_all[:], orep, pre_scale)

            resident = {}

            # ---------------- sweep 1: u_hat + S0 ----------------
            with tc.tile_pool(name="xw", bufs=1) as xw:
                xbd_sb = xw.tile([128, NP, 128], f16)
                nc.sync.dma_start(xbd_sb[:], x_bd[:])
                subacc = [xw.tile([128, DN], f16, tag=f"sa{k}", name=f"sa{k}")
                          for k in range(8)]
                with tc.tile_pool(name="wp", bufs=6) as wp, \
                     tc.tile_pool(name="u16s1", bufs=10) as u16s1, \
                     tc.tile_pool(name="psum1", bufs=4, space="PSUM") as psum1:
                    for gp in range(NP):
                        wt = wp.tile([128, DN], f16)
                        nc.sync.dma_start(wt[:], w_t2[gp])
                        for gs in range(2):
                            g = 2 * gp + gs
                            u16 = u16s1.tile([128, DN], f16, tag="us1",
                                             name=f"us1_{g}")[:]
                            for h in range(2):
                                pu = psum1.tile([128, DN // 2], f32)
                                for ch in range(2):
                                    nc.tensor.matmul(
                                        pu[:, 512 * ch:512 * (ch + 1)],
                                        lhsT=xbd_sb[64 * gs:64 * (gs + 1),
                                                    gp, :],
                                        rhs=wt[64 * gs:64 * (gs + 1),
                                               1024 * h + 512 * ch:
                                               1024 * h + 512 * (ch + 1)],
                                        start=True, stop=True)
                                # drain: 3/4 of groups on ACT, 1/4 on DVE
                                if g % 4 == 3:
                                    nc.vector.tensor_copy(
                                        u16[:, 1024 * h:1024 * (h + 1)],
                                        pu[:])
                                else:
                                    nc.scalar.copy(
                                        u16[:, 1024 * h:1024 * (h + 1)],
                                        pu[:])
                            nc.sync.dma_start(u_store[g], u16)
                            # accumulate S0 on DVE in f16 sub-accumulators
                            sa = subacc[g // 8]
                            if g % 8 == 0:
                                nc.vector.tensor_copy(sa[:], u16)
                            else:
                                nc.vector.tensor_add(sa[:], sa[:], u16)

                # merge sub-accumulators (f16), fold j-slots via matmul
                for a, b_ in [(0, 1), (2, 3), (4, 5), (6, 7), (0, 2),
                              (4, 6), (0, 4)]:
                    nc.vector.tensor_add(subacc[a][:], subacc[a][:],
                                         subacc[b_][:])
                psacc = _late.enter_context(
                    tc.tile_pool(name="psacc", bufs=1, space="PSUM"))
                s0_ps = psacc.tile([B, DN], f32, tag="sacc")
                for ch in range(4):
                    nc.tensor.matmul(
                        s0_ps[:, 512 * ch:512 * (ch + 1)],
                        lhsT=ones_sb[:],
                        rhs=subacc[0][:, 512 * ch:512 * (ch + 1)],
                        start=True, stop=True)

                # S0 exchange + squash -> out_rep[0]
                s0_dr = tail.tile([B, DN], f16, tag="t_drain")
                nc.scalar.copy(s0_dr[:], s0_ps[:])
                exchange_and_squash(s0_dr[:], 0, out_rep[0], 1.0 / 64.0)

            # ---------------- sweeps 2 and 3: routing ----------------
            # b-state: [128, NP, 2, N] f32 mega-tile persistent across sweeps
            bs = bstate.tile([128, NP, 2, N], f32)
            # processing order: resident pairs first (both sweeps use the
            # same order; bs is indexed by position, u_store by gp)
            order = list(range(NP))
            with tc.tile_pool(name="u16p", bufs=9) as u16p, \
                 tc.tile_pool(name="big", bufs=2) as big, \
                 tc.tile_pool(name="tree", bufs=2) as tree, \
                 tc.tile_pool(name="soft", bufs=2) as soft:
                for it in range(2):
                    s_ps = psacc.tile([B, DN], f32, tag="sacc")
                    first_mm = True
                    for w in range(NP // WIN):
                        u16s = []
                        t4 = soft.tile([128, WIN, 2, 4, N], f16, tag="t4")
                        for pw in range(WIN):
                            gp = order[WIN * w + pw]
                            if it == 0 and gp in resident:
                                u16 = resident[gp]
                            else:
                                u16 = u16p.tile([128, 2, DN], f16,
                                                tag="u16",
                                                name=f"u{it}_{gp}")
                                nc.sync.dma_start(
                                    u16[:],
                                    u_store[2 * gp:2 * gp + 2]
                                    .transpose([1, 0, 2]))
                            u16s.append(u16)
                            u4 = u16[:].rearrange("p a (d n) -> p a d n",
                                                  n=N)
                            orep4 = (out_rep[it][:]
                                     .rearrange("p (d n) -> p d n", n=N)
                                     .unsqueeze(1)
                                     .broadcast_to([128, 2, D, N]))
                            tmp = big.tile([128, 2, D, N], f16, tag="tmp")
                            nc.vector.tensor_mul(tmp[:], u4, orep4)
                            # per-pair tree down to d=4, last level lands
                            # in the shared window tile t4
                            cur, d = tmp, D
                            while d > 8:
                                nxt = tree.tile([128, 2, d // 2, N], f16,
                                                tag=f"tr{d}")
                                nc.vector.tensor_add(
                                    nxt[:], cur[:, :, 0:d // 2, :],
                                    cur[:, :, d // 2:d, :])
                                cur, d = nxt, d // 2
                            nc.vector.tensor_add(
                                t4[:, pw], cur[:, :, 0:4, :],
                                cur[:, :, 4:8, :])

                        # batched tree tail + agreement for the window
                        bsw = bs[:, WIN * w:WIN * (w + 1), :, :]
                        t2 = soft.tile([128, WIN, 2, 2, N], f16, tag="t2")
                        nc.vector.tensor_add(t2[:], t4[:, :, :, 0:2, :],
                                             t4[:, :, :, 2:4, :])
                        if it == 0:
                            nc.vector.tensor_add(
                                bsw, t2[:, :, :, 0, :], t2[:, :, :, 1, :])
                        else:
                            a2b = soft.tile([128, WIN, 2, N], f32,
                                            tag="a2b")
                            nc.vector.tensor_add(
                                a2b[:], t2[:, :, :, 0, :],
                                t2[:, :, :, 1, :])
                            nc.vector.tensor_add(bsw, bsw, a2b[:])

                        # batched softmax for window w (WIN pairs)
                        m8 = soft.tile([128, WIN * 2], f32, tag="m8")
                        nc.vector.tensor_reduce(
                            out=m8[:].rearrange("p (a c) -> p a c", c=2)
                                     .unsqueeze(-1),
                            in_=bsw, axis=AX.X, op=OP.max)
                        nm8 = soft.tile([128, WIN * 2], f32, tag="nm8")
                        nc.vector.tensor_scalar_mul(nm8[:], m8[:],
                                                    -INV_LOG2)
                        e8 = soft.tile([128, WIN, 2, N], f16, tag="e8")
                        z8 = soft.tile([128, WIN * 2], f32, tag="z8")
                        for pw in range(WIN):
                            for gs in range(2):
                                k = 2 * pw + gs
                                nc.scalar.activation(
                                    e8[:, pw, gs, :],
                                    bs[:, WIN * w + pw, gs, :],
                                    AF.Exp, bias=nm8[:, k:k + 1],
                                    scale=INV_LOG2,
                                    accum_out=z8[:, k:k + 1])
                        rz8 = soft.tile([128, WIN * 2], f32, tag="rz8")
                        nc.vector.reciprocal(rz8[:], z8[:])
                        # 1/Z folded into the fold-matmul lhsT via ACT
                        cz = soft.tile([128, WIN, 2, 32], f16, tag="cz")
                        for pw in range(WIN):
                            for gs in range(2):
                                k = 2 * pw + gs
                                nc.scalar.activation(
                                    cz[:, pw, gs, :], ones_sb[:],
                                    AF.Copy, bias=0.0,
                                    scale=rz8[:, k:k + 1])

                        # weight-apply + fold per pair
                        for pw in range(WIN):
                            pos = WIN * w + pw
                            u16 = u16s[pw]
                            u4 = u16[:].rearrange("p a (d n) -> p a d n",
                                                  n=N)
                            erep = (e8[:, pw, :, :].unsqueeze(2)
                                    .broadcast_to([128, 2, D, N]))
                            sm = big.tile([128, 2, D, N], f16, tag="sm")
                            nc.vector.tensor_mul(sm[:], u4, erep)
                            smf = sm[:].rearrange("p a d n -> p a (d n)")
                            for gs in range(2):
                                for ch in range(4):
                                    nc.tensor.matmul(
                                        s_ps[:, 512 * ch:512 * (ch + 1)],
                                        lhsT=cz[:, pw, gs, :],
                                        rhs=smf[:, gs,
                                                512 * ch:512 * (ch + 1)],
                                        start=first_mm,
                                        stop=(pos == NP - 1 and gs == 1),
                                        skip_group_check=True)
                                first_mm = False

                    if it == 0:
                        s_sb = tail.tile([B, DN], f16, tag="t_drain")
                        nc.scalar.copy(s_sb[:], s_ps[:])
                        exchange_and_squash(s_sb[:], 1, out_rep[1], 1.0)
                    else:
                        s_f = tail.tile([B, DN], f32, tag="t_fin")
                        nc.scalar.copy(s_f[:], s_ps[:])
                        nc.sync.dma_start(s2_part[:], s_f[:])
            _late.close()

    nc.compile()
    return nc


_NC_CACHE = {}


def _get_nc():
    if "nc" not in _NC_CACHE:
        _NC_CACHE["nc"] = _build()
    return _NC_CACHE["nc"]


def _prep_core(x_c, w_c):
    """x_c [B, IL, K] f32, w_c [N, IL, D, K] f32 -> in_map dict."""
    wt = np.ascontiguousarray(w_c.transpose(1, 3, 2, 0))  # [IL, K, D, N]
    wt2 = wt.reshape(NP, 8, K, DN).reshape(NP, 128, DN).astype(np.float16)
    xt = x_c.transpose(1, 2, 0)  # [IL, K, B]
    x_bd = np.zeros((128, NP, 128), np.float16)
    for g in range(G):
        q, s = g // 2, g % 2
        for j in range(4):
            i = 4 * g + j
            x_bd[s * 64 + j * 16:s * 64 + j * 16 + K, q,
                 j * 32:j * 32 + 32] = xt[i].astype(np.float16)
    return {"w_t2": wt2, "x_bd": x_bd}


def _squash_np(v):
    sn = np.sum(v * v, axis=-1, keepdims=True)
    return np.sqrt(sn) / (1.0 + sn) * v


def _run(inputs, W, trace=False):
    _install_ntff_hook()
    nc = _get_nc()
    x = np.asarray(inputs, np.float32)
    Wf = np.asarray(W, np.float32)
    in_maps = []
    for c in range(NCORES):
        sl = slice(c * IL, (c + 1) * IL)
        in_maps.append(_prep_core(x[:, sl, :], Wf[:, sl, :, :]))
    res = run_bass_kernel_spmd(nc, in_maps, list(range(NCORES)), trace=trace)
    s2 = np.zeros((B, DN), np.float64)
    for c in range(NCORES):
        s2 += res.results[c]["s2_part"].astype(np.float64)
    s2 = s2.reshape(B, D, N).transpose(0, 2, 1).astype(np.float32)
    out = _squash_np(s2).astype(np.float32)
    return out, res


def kernel(inputs, W):
    out, _ = _run(inputs, W, trace=False)
    return out


# revision 70
# speedup vs baseline: 1.0011x; 1.0011x over previous
"""Capsule-routing kernel for 8 Trainium2 NeuronCores.

Problem: u_hat = einsum('nidk,bik->bnid', W, x); 3 rounds of dynamic
routing (softmax over n, weighted sum over i, squash, agreement update).

Sharding: input-capsule axis i (2048) split 8 ways -> 256 i per core.
Softmax over n is local; the per-iteration weighted sum s[b,n,d] is a
partial over local i, combined with an on-device AllReduce (iterations
1,2) or on the host (final iteration).

Per-core schedule (B=32, N=64, IL=256, D=32, K=16):
  sweep 1: u_hat via TensorE (block-diag x lhsT, K=64, M=128 -> psum
           [(j,b), (d,n)]), drain-cast to fp16 split between ACT and
           DVE, store to DRAM; S0 accumulated on PE via an accumulating
           ones-matmul (no DVE subaccs). AllReduce S0 (f16 payload),
           squash -> out0 [128, 2048] f16 (partition-replicated x4).
  sweep 2/3 (per pair of 4-i groups): load u16 [128,2,2048], DVE
           tmp=u16*out_rep (one 2x op, broadcast middle dim),
           halving-tree over d -> agreement into a [128,NP,2,N] f32
           mega-state; per 4-pair window: batched max-reduce, stt
           max-subtract+INV_LOG2 scale, one ACT exp, batched Z-reduce,
           reciprocal, e_norm = e*(1/Z); sm = u16*e_norm_rep in ONE 2x
           DVE op (broadcast over d); fold with constant block-ones
           lhsT accumulating s partial in psum (PE). AllReduce+squash
           between sweeps; final partial summed+squashed on host.

Layouts: u16 partition p = 32*j + b (j = i mod 4 within group), free
(d,n) d-major so d-reductions and e/out broadcasts keep the packed
last dim (n) required for DVE 2x mode.
"""
import sys
import types

sys.path.insert(0, "/opt/trn_rl_repo")

import numpy as np

from concourse import bacc, tile, mybir
from concourse.bass_utils import run_bass_kernel_spmd

f32 = mybir.dt.float32
f16 = mybir.dt.float16
AX = mybir.AxisListType
OP = mybir.AluOpType
AF = mybir.ActivationFunctionType

B, N, I, D, K = 32, 64, 2048, 32, 16
NCORES = 8
IL = I // NCORES          # 256 local input capsules
G = IL // 4               # 64 groups of 4 i
NP = G // 2               # 32 group-pairs
DN = D * N                # 2048 free elements per group, d-major
INV_LOG2 = float(1.0 / np.log(2.0))
WIN = 4                   # group-pairs per softmax batch window


def _install_ntff_hook():
    if "antenv.axon_hooks" in sys.modules:
        return
    try:
        mod = types.ModuleType("antenv.axon_hooks")
        state = {"hook": None}
        mod.set_axon_ntff_profile_hook = lambda h: state.__setitem__("hook", h)
        mod.get_axon_ntff_profile_hook = lambda: state["hook"]
        sys.modules["antenv.axon_hooks"] = mod
        import antenv
        antenv.axon_hooks = mod
        from trn_agent_boot.trn_boot import _ntff_profile_via_ctypes
        mod.set_axon_ntff_profile_hook(
            _ntff_profile_via_ctypes("/opt/axon/libaxon_pjrt.so"))
    except Exception:
        pass


def _build():
    nc = bacc.Bacc("TRN2", target_bir_lowering=False, debug=False,
                   num_devices=NCORES)

    w_t2 = nc.dram_tensor("w_t2", [NP, 128, DN], f16, kind="ExternalInput")
    x_bd = nc.dram_tensor("x_bd", [128, NP, 128], f16, kind="ExternalInput")
    s2_part = nc.dram_tensor("s2_part", [B, DN], f32, kind="ExternalOutput")

    u_store = nc.dram_tensor("u_store", [G, 128, DN], f16)
    cc_in = [nc.dram_tensor(f"cc_in{r}", [B, DN], f16) for r in range(2)]
    cc_out = [nc.dram_tensor(f"cc_out{r}", [B, DN], f16, addr_space="Shared")
              for r in range(2)]
    # tiny warm-up collective: absorbs ncfw first-call staging while
    # sweep 1 computes (collectives run on TOPSP, not the 5 engines)
    cc_win = nc.dram_tensor("cc_win", [32, 16], f16)
    cc_wout = nc.dram_tensor("cc_wout", [32, 16], f16, addr_space="Shared")

    ones4_np = np.zeros((128, 32), np.float16)
    for p in range(128):
        ones4_np[p, p % 32] = 1.0
    ones4 = nc.inline_tensor(ones4_np, name="ones4")

    core_ids = list(range(NCORES))

    with tile.TileContext(nc) as tc:
        from contextlib import ExitStack
        _late = ExitStack()
        with tc.tile_pool(name="const", bufs=1) as constp, \
             tc.tile_pool(name="tail", bufs=1) as tail, \
             tc.tile_pool(name="small", bufs=4) as small, \
             tc.tile_pool(name="bstate", bufs=1) as bstate:

            ones_sb = constp.tile([128, 32], f16)
            nc.sync.dma_start(ones_sb[:], ones4[:])
            out_rep = [constp.tile([128, DN], f16, tag=f"orep{r}",
                                   name=f"orep{r}") for r in range(2)]

            # warm-up collectives, overlapped with sweep-1 startup: each
            # ncfw collective runs faster than the previous, so two dummies
            # make both real AllReduces hit the warmed path
            wtile = constp.tile([32, 16], f16, tag="warm")
            nc.vector.memset(wtile[:], 1.0)
            nc.sync.dma_start(cc_win[:], wtile[:])
            for _ in range(2):
                nc.gpsimd.collective_compute(
                    "AllReduce", OP.add, ins=[cc_win[:]],
                    outs=[cc_wout[:]], replica_groups=[core_ids])

            def squash_to_outrep(s_sb, orep, pre_scale):
                """orep [128, (d,n)] f16 <- x4-replicated squash(s_sb*pre_scale).
                s_sb is a [B, DN] f16 AP."""
                ps2 = float(pre_scale * pre_scale)
                s3 = s_sb.rearrange("p (d n) -> p d n", n=N)
                sq = tail.tile([32, D, N], f32, tag="t_sq")
                nc.vector.tensor_mul(sq[:], s3, s3)
                cur, d = sq, D
                while d > 2:
                    nxt = tail.tile([32, d // 2, N], f32, tag=f"t_tr{d}")
                    nc.vector.tensor_add(nxt[:], cur[:, 0:d // 2, :],
                                         cur[:, d // 2:d, :])
                    cur, d = nxt, d // 2
                sn = tail.tile([32, 1, N], f32, tag="t_sn")
                nc.vector.tensor_add(sn[:], cur[:, 0:1, :], cur[:, 1:2, :])
                r_ = tail.tile([32, N], f32, tag="t_r")
                nc.scalar.activation(r_[:], sn[:, 0, :], AF.Sqrt,
                                     bias=0.0, scale=ps2)
                den = tail.tile([32, N], f32, tag="t_den")
                nc.vector.tensor_scalar(den[:], sn[:, 0, :], ps2, 1.0,
                                        OP.mult, OP.add)
                rd = tail.tile([32, N], f32, tag="t_rd")
                nc.vector.reciprocal(rd[:], den[:])
                fac = tail.tile([32, N], f16, tag="t_fac")
                nc.vector.scalar_tensor_tensor(fac[:], r_[:],
                                               float(pre_scale), rd[:],
                                               op0=OP.mult, op1=OP.mult)
                o16 = tail.tile([32, D, N], f16, tag="t_o16")
                nc.vector.tensor_mul(
                    o16[:], s3,
                    fac[:].unsqueeze(1).broadcast_to([32, D, N]))
                for j in range(4):
                    nc.sync.dma_start(
                        orep[32 * j:32 * j + 32, :],
                        o16[:].rearrange("p d n -> p (d n)"))

            def exchange_and_squash(s_sb16, r, orep, pre_scale):
                """AllReduce the [B, DN] f16 partial, squash into orep."""
                nc.sync.dma_start(cc_in[r][:], s_sb16[:])
                nc.gpsimd.collective_compute(
                    "AllReduce", OP.add, ins=[cc_in[r][:]],
                    outs=[cc_out[r][:]], replica_groups=[core_ids])
                s_all = tail.tile([B, DN], f16, tag="t_all")
                nc.sync.dma_start(s_all[:], cc_out[r][:])
                squash_to_outrep(s_all[:], orep, pre_scale)

            # u16 pool reserved before sweep 1 (validated geometry)
            u16p = _late.enter_context(tc.tile_pool(name="u16p", bufs=9))
            resident = {}

            # ---------------- sweep 1: u_hat + S0 ----------------
            with tc.tile_pool(name="xw", bufs=1) as xw:
                xbd_sb = xw.tile([128, NP, 128], f16)
                nc.sync.dma_start(xbd_sb[:], x_bd[:])
                subacc = [xw.tile([128, DN], f16, tag=f"sa{k}", name=f"sa{k}")
                          for k in range(8)]
                with tc.tile_pool(name="wp", bufs=4) as wp, \
                     tc.tile_pool(name="u16s1", bufs=4) as u16s1, \
                     tc.tile_pool(name="psum1", bufs=4, space="PSUM") as psum1:
                    for gp in range(NP):
                        wt = wp.tile([128, DN], f16)
                        nc.sync.dma_start(wt[:], w_t2[gp])
                        for gs in range(2):
                            g = 2 * gp + gs
                            u16 = u16s1.tile([128, DN], f16, tag="us1",
                                             name=f"us1_{g}")[:]
                            for h in range(2):
                                pu = psum1.tile([128, DN // 2], f32)
                                for ch in range(2):
                                    nc.tensor.matmul(
                                        pu[:, 512 * ch:512 * (ch + 1)],
                                        lhsT=xbd_sb[64 * gs:64 * (gs + 1),
                                                    gp, :],
                                        rhs=wt[64 * gs:64 * (gs + 1),
                                               1024 * h + 512 * ch:
                                               1024 * h + 512 * (ch + 1)],
                                        start=True, stop=True)
                                # drain: 3/4 of groups on ACT, 1/4 on DVE
                                if g % 4 == 3:
                                    nc.vector.tensor_copy(
                                        u16[:, 1024 * h:1024 * (h + 1)],
                                        pu[:])
                                else:
                                    nc.scalar.copy(
                                        u16[:, 1024 * h:1024 * (h + 1)],
                                        pu[:])
                            nc.sync.dma_start(u_store[g], u16)
                            # accumulate S0 on DVE in f16 sub-accumulators
                            sa = subacc[g // 8]
                            if g % 8 == 0:
                                nc.vector.tensor_copy(sa[:], u16)
                            else:
                                nc.vector.tensor_add(sa[:], sa[:], u16)

                # merge sub-accumulators (f16), fold j-slots via matmul
                for a, b_ in [(0, 1), (2, 3), (4, 5), (6, 7), (0, 2),
                              (4, 6), (0, 4)]:
                    nc.vector.tensor_add(subacc[a][:], subacc[a][:],
                                         subacc[b_][:])
                psacc = _late.enter_context(
                    tc.tile_pool(name="psacc", bufs=1, space="PSUM"))
                s0_ps = psacc.tile([B, DN], f32, tag="sacc")
                for ch in range(4):
                    nc.tensor.matmul(
                        s0_ps[:, 512 * ch:512 * (ch + 1)],
                        lhsT=ones_sb[:],
                        rhs=subacc[0][:, 512 * ch:512 * (ch + 1)],
                        start=True, stop=True)

                # S0 exchange + squash -> out_rep[0]
                s0_dr = tail.tile([B, DN], f16, tag="t_drain")
                nc.scalar.copy(s0_dr[:], s0_ps[:])
                exchange_and_squash(s0_dr[:], 0, out_rep[0], 1.0 / 64.0)

            # ---------------- sweeps 2 and 3: routing ----------------
            # b-state: [128, NP, 2, N] f32 mega-tile persistent across sweeps
            bs = bstate.tile([128, NP, 2, N], f32)
            # processing order: resident pairs first (both sweeps use the
            # same order; bs is indexed by position, u_store by gp)
            order = list(range(NP))
            with tc.tile_pool(name="big", bufs=2) as big, \
                 tc.tile_pool(name="tree", bufs=2) as tree, \
                 tc.tile_pool(name="soft", bufs=2) as soft:
                for it in range(2):
                    s_ps = psacc.tile([B, DN], f32, tag="sacc")
                    first_mm = True
                    for w in range(NP // WIN):
                        u16s = []
                        t4 = soft.tile([128, WIN, 2, 4, N], f16, tag="t4")
                        for pw in range(WIN):
                            gp = order[WIN * w + pw]
                            if it == 0 and gp in resident:
                                u16 = resident[gp]
                            else:
                                u16 = u16p.tile([128, 2, DN], f16,
                                                tag="u16",
                                                name=f"u{it}_{gp}")
                                nc.sync.dma_start(
                                    u16[:],
                                    u_store[2 * gp:2 * gp + 2]
                                    .transpose([1, 0, 2]))
                            u16s.append(u16)
                            u4 = u16[:].rearrange("p a (d n) -> p a d n",
                                                  n=N)
                            orep4 = (out_rep[it][:]
                                     .rearrange("p (d n) -> p d n", n=N)
                                     .unsqueeze(1)
                                     .broadcast_to([128, 2, D, N]))
                            tmp = big.tile([128, 2, D, N], f16, tag="tmp")
                            nc.vector.tensor_mul(tmp[:], u4, orep4)
                            # per-pair tree down to d=4, last level lands
                            # in the shared window tile t4
                            cur, d = tmp, D
                            while d > 8:
                                nxt = tree.tile([128, 2, d // 2, N], f16,
                                                tag=f"tr{d}")
                                nc.vector.tensor_add(
                                    nxt[:], cur[:, :, 0:d // 2, :],
                                    cur[:, :, d // 2:d, :])
                                cur, d = nxt, d // 2
                            nc.vector.tensor_add(
                                t4[:, pw], cur[:, :, 0:4, :],
                                cur[:, :, 4:8, :])

                        # batched tree tail + agreement for the window
                        bsw = bs[:, WIN * w:WIN * (w + 1), :, :]
                        t2 = soft.tile([128, WIN, 2, 2, N], f16, tag="t2")
                        nc.vector.tensor_add(t2[:], t4[:, :, :, 0:2, :],
                                             t4[:, :, :, 2:4, :])
                        if it == 0:
                            nc.vector.tensor_add(
                                bsw, t2[:, :, :, 0, :], t2[:, :, :, 1, :])
                        else:
                            a2b = soft.tile([128, WIN, 2, N], f32,
                                            tag="a2b")
                            nc.vector.tensor_add(
                                a2b[:], t2[:, :, :, 0, :],
                                t2[:, :, :, 1, :])
                            nc.vector.tensor_add(bsw, bsw, a2b[:])

                        # batched softmax for window w (WIN pairs)
                        m8 = soft.tile([128, WIN * 2], f32, tag="m8")
                        nc.vector.tensor_reduce(
                            out=m8[:].rearrange("p (a c) -> p a c", c=2)
                                     .unsqueeze(-1),
                            in_=bsw, axis=AX.X, op=OP.max)
                        nm8 = soft.tile([128, WIN * 2], f32, tag="nm8")
                        nc.vector.tensor_scalar_mul(nm8[:], m8[:],
                                                    -INV_LOG2)
                        e8 = soft.tile([128, WIN, 2, N], f16, tag="e8")
                        z8 = soft.tile([128, WIN * 2], f32, tag="z8")
                        for pw in range(WIN):
                            for gs in range(2):
                                k = 2 * pw + gs
                                nc.scalar.activation(
                                    e8[:, pw, gs, :],
                                    bs[:, WIN * w + pw, gs, :],
                                    AF.Exp, bias=nm8[:, k:k + 1],
                                    scale=INV_LOG2,
                                    accum_out=z8[:, k:k + 1])
                        rz8 = soft.tile([128, WIN * 2], f32, tag="rz8")
                        nc.vector.reciprocal(rz8[:], z8[:])
                        # 1/Z folded into the fold-matmul lhsT via ACT
                        cz = soft.tile([128, WIN, 2, 32], f16, tag="cz")
                        for pw in range(WIN):
                            for gs in range(2):
                                k = 2 * pw + gs
                                nc.scalar.activation(
                                    cz[:, pw, gs, :], ones_sb[:],
                                    AF.Copy, bias=0.0,
                                    scale=rz8[:, k:k + 1])

                        # weight-apply + fold per pair
                        for pw in range(WIN):
                            pos = WIN * w + pw
                            u16 = u16s[pw]
                            u4 = u16[:].rearrange("p a (d n) -> p a d n",
                                                  n=N)
                            erep = (e8[:, pw, :, :].unsqueeze(2)
                                    .broadcast_to([128, 2, D, N]))
                            sm = big.tile([128, 2, D, N], f16, tag="sm")
                            nc.vector.tensor_mul(sm[:], u4, erep)
                            smf = sm[:].rearrange("p a d n -> p a (d n)")
                            for gs in range(2):
                                for ch in range(4):
                                    nc.tensor.matmul(
                                        s_ps[:, 512 * ch:512 * (ch + 1)],
                                        lhsT=cz[:, pw, gs, :],
                                        rhs=smf[:, gs,
                                                512 * ch:512 * (ch + 1)],
                                        start=first_mm,
                                        stop=(pos == NP - 1 and gs == 1),
                                        skip_group_check=True)
                                first_mm = False

                    if it == 0:
                        s_sb = tail.tile([B, DN], f16, tag="t_drain")
                        nc.scalar.copy(s_sb[:], s_ps[:])
                        exchange_and_squash(s_sb[:], 1, out_rep[1], 1.0)
                    else:
                        s_f = tail.tile([B, DN], f32, tag="t_fin")
                        nc.scalar.copy(s_f[:], s_ps[:])
                        nc.sync.dma_start(s2_part[:], s_f[:])
            _late.close()

    nc.compile()
    return nc


_NC_CACHE = {}


def _get_nc():
    if "nc" not in _NC_CACHE:
        _NC_CACHE["nc"] = _build()
    return _NC_CACHE["nc"]


def _prep_core(x_c, w_c):
    """x_c [B, IL, K] f32, w_c [N, IL, D, K] f32 -> in_map dict."""
    wt = np.ascontiguousarray(w_c.transpose(1, 3, 2, 0))  # [IL, K, D, N]
    wt2 = wt.reshape(NP, 8, K, DN).reshape(NP, 128, DN).astype(np.float16)
    xt = x_c.transpose(1, 2, 0)  # [IL, K, B]
    x_bd = np.zeros((128, NP, 128), np.float16)
    for g in range(G):
        q, s = g // 2, g % 2
        for j in range(4):
            i = 4 * g + j
            x_bd[s * 64 + j * 16:s * 64 + j * 16 + K, q,
                 j * 32:j * 32 + 32] = xt[i].astype(np.float16)
    return {"w_t2": wt2, "x_bd": x_bd}


def _squash_np(v):
    sn = np.sum(v * v, axis=-1, keepdims=True)
    return np.sqrt(sn) / (1.0 + sn) * v


def _run(inputs, W, trace=False):
    _install_ntff_hook()
    nc = _get_nc()
    x = np.asarray(inputs, np.float32)
    Wf = np.asarray(W, np.float32)
    in_maps = []
    for c in range(NCORES):
        sl = slice(c * IL, (c + 1) * IL)
        in_maps.append(_prep_core(x[:, sl, :], Wf[:, sl, :, :]))
    res = run_bass_kernel_spmd(nc, in_maps, list(range(NCORES)), trace=trace)
    s2 = np.zeros((B, DN), np.float64)
    for c in range(NCORES):
        s2 += res.results[c]["s2_part"].astype(np.float64)
    s2 = s2.reshape(B, D, N).transpose(0, 2, 1).astype(np.float32)
    out = _squash_np(s2).astype(np.float32)
    return out, res


def kernel(inputs, W):
    out, _ = _run(inputs, W, trace=False)
    return out


# revision 72
# speedup vs baseline: 1.1421x; 1.1408x over previous
"""Capsule-routing kernel for 8 Trainium2 NeuronCores.

Problem: u_hat = einsum('nidk,bik->bnid', W, x); 3 rounds of dynamic
routing (softmax over n, weighted sum over i, squash, agreement update).

Sharding: input-capsule axis i (2048) split 8 ways -> 256 i per core.
Softmax over n is local; the per-iteration weighted sum s[b,n,d] is a
partial over local i, combined with an on-device AllReduce (iterations
1,2) or on the host (final iteration).

Per-core schedule (B=32, N=64, IL=256, D=32, K=16):
  sweep 1: u_hat via TensorE (block-diag x lhsT, K=64, M=128 -> psum
           [(j,b), (d,n)]), drain-cast to fp16 split between ACT and
           DVE, store to DRAM; S0 accumulated on PE via an accumulating
           ones-matmul (no DVE subaccs). AllReduce S0 (f16 payload),
           squash -> out0 [128, 2048] f16 (partition-replicated x4).
  sweep 2/3 (per pair of 4-i groups): load u16 [128,2,2048], DVE
           tmp=u16*out_rep (one 2x op, broadcast middle dim),
           halving-tree over d -> agreement into a [128,NP,2,N] f32
           mega-state; per 4-pair window: batched max-reduce, stt
           max-subtract+INV_LOG2 scale, one ACT exp, batched Z-reduce,
           reciprocal, e_norm = e*(1/Z); sm = u16*e_norm_rep in ONE 2x
           DVE op (broadcast over d); fold with constant block-ones
           lhsT accumulating s partial in psum (PE). AllReduce+squash
           between sweeps; final partial summed+squashed on host.

Layouts: u16 partition p = 32*j + b (j = i mod 4 within group), free
(d,n) d-major so d-reductions and e/out broadcasts keep the packed
last dim (n) required for DVE 2x mode.
"""
import sys
import types

sys.path.insert(0, "/opt/trn_rl_repo")

import numpy as np

from concourse import bacc, tile, mybir
from concourse.bass_utils import run_bass_kernel_spmd

f32 = mybir.dt.float32
f16 = mybir.dt.float16
AX = mybir.AxisListType
OP = mybir.AluOpType
AF = mybir.ActivationFunctionType

B, N, I, D, K = 32, 64, 2048, 32, 16
NCORES = 8
IL = I // NCORES          # 256 local input capsules
G = IL // 4               # 64 groups of 4 i
NP = G // 2               # 32 group-pairs
DN = D * N                # 2048 free elements per group, d-major
INV_LOG2 = float(1.0 / np.log(2.0))
WIN = 4                   # group-pairs per softmax batch window


def _install_ntff_hook():
    if "antenv.axon_hooks" in sys.modules:
        return
    try:
        mod = types.ModuleType("antenv.axon_hooks")
        state = {"hook": None}
        mod.set_axon_ntff_profile_hook = lambda h: state.__setitem__("hook", h)
        mod.get_axon_ntff_profile_hook = lambda: state["hook"]
        sys.modules["antenv.axon_hooks"] = mod
        import antenv
        antenv.axon_hooks = mod
        from trn_agent_boot.trn_boot import _ntff_profile_via_ctypes
        mod.set_axon_ntff_profile_hook(
            _ntff_profile_via_ctypes("/opt/axon/libaxon_pjrt.so"))
    except Exception:
        pass


def _build():
    nc = bacc.Bacc("TRN2", target_bir_lowering=False, debug=False,
                   num_devices=NCORES)

    w_t2 = nc.dram_tensor("w_t2", [NP, 128, DN], f16, kind="ExternalInput")
    x_bd = nc.dram_tensor("x_bd", [128, NP, 128], f16, kind="ExternalInput")
    s2_part = nc.dram_tensor("s2_part", [B, DN], f32, kind="ExternalOutput")

    u_store = nc.dram_tensor("u_store", [G, 128, DN], f16)
    cc_in = [nc.dram_tensor(f"cc_in{r}", [B, DN], f16) for r in range(2)]
    cc_out = [nc.dram_tensor(f"cc_out{r}", [B, DN], f16, addr_space="Shared")
              for r in range(2)]
    # tiny warm-up collective: absorbs ncfw first-call staging while
    # sweep 1 computes (collectives run on TOPSP, not the 5 engines)
    cc_win = nc.dram_tensor("cc_win", [32, 16], f16)
    cc_wout = nc.dram_tensor("cc_wout", [32, 16], f16, addr_space="Shared")

    ones4_np = np.zeros((128, 32), np.float16)
    for p in range(128):
        ones4_np[p, p % 32] = 1.0
    ones4 = nc.inline_tensor(ones4_np, name="ones4")

    core_ids = list(range(NCORES))

    with tile.TileContext(nc) as tc:
        from contextlib import ExitStack
        _late = ExitStack()
        with tc.tile_pool(name="const", bufs=1) as constp, \
             tc.tile_pool(name="tail", bufs=1) as tail, \
             tc.tile_pool(name="small", bufs=4) as small, \
             tc.tile_pool(name="bstate", bufs=1) as bstate:

            ones_sb = constp.tile([128, 32], f16)
            nc.sync.dma_start(ones_sb[:], ones4[:])
            out_rep = [constp.tile([128, DN], f16, tag=f"orep{r}",
                                   name=f"orep{r}") for r in range(2)]

            # warm-up collectives, overlapped with sweep-1 startup: each
            # ncfw collective runs faster than the previous, so two dummies
            # make both real AllReduces hit the warmed path
            wtile = constp.tile([32, 16], f16, tag="warm")
            nc.vector.memset(wtile[:], 1.0)
            nc.sync.dma_start(cc_win[:], wtile[:])
            for _ in range(2):
                nc.gpsimd.collective_compute(
                    "AllReduce", OP.add, ins=[cc_win[:]],
                    outs=[cc_wout[:]], replica_groups=[core_ids])

            def squash_to_outrep(s_sb, orep, pre_scale):
                """orep [128, (d,n)] f16 <- x4-replicated squash(s_sb*pre_scale).
                s_sb is a [B, DN] f16 AP."""
                ps2 = float(pre_scale * pre_scale)
                s3 = s_sb.rearrange("p (d n) -> p d n", n=N)
                # r=1 (pre_scale=1, |s|<~10): f16 square+tree is safe
                # (sum_d s^2 <= ~3e3) and runs at DVE 2x; r=0's unscaled
                # s0 would overflow f16 when squared -> keep f32 there
                adt = f16 if pre_scale == 1.0 else f32
                sq = tail.tile([32, D, N], adt, tag="t_sq")
                nc.vector.tensor_mul(sq[:], s3, s3)
                cur, d = sq, D
                while d > 2:
                    nxt = tail.tile([32, d // 2, N], adt, tag=f"t_tr{d}")
                    nc.vector.tensor_add(nxt[:], cur[:, 0:d // 2, :],
                                         cur[:, d // 2:d, :])
                    cur, d = nxt, d // 2
                sn = tail.tile([32, 1, N], f32, tag="t_sn")
                nc.vector.tensor_add(sn[:], cur[:, 0:1, :], cur[:, 1:2, :])
                r_ = tail.tile([32, N], f32, tag="t_r")
                nc.scalar.activation(r_[:], sn[:, 0, :], AF.Sqrt,
                                     bias=0.0, scale=ps2)
                den = tail.tile([32, N], f32, tag="t_den")
                nc.vector.tensor_scalar(den[:], sn[:, 0, :], ps2, 1.0,
                                        OP.mult, OP.add)
                rd = tail.tile([32, N], f32, tag="t_rd")
                nc.vector.reciprocal(rd[:], den[:])
                fac = tail.tile([32, N], f16, tag="t_fac")
                nc.vector.scalar_tensor_tensor(fac[:], r_[:],
                                               float(pre_scale), rd[:],
                                               op0=OP.mult, op1=OP.mult)
                o16 = tail.tile([32, D, N], f16, tag="t_o16")
                nc.vector.tensor_mul(
                    o16[:], s3,
                    fac[:].unsqueeze(1).broadcast_to([32, D, N]))
                for j in range(4):
                    nc.sync.dma_start(
                        orep[32 * j:32 * j + 32, :],
                        o16[:].rearrange("p d n -> p (d n)"))

            def exchange_and_squash(s_sb16, r, orep, pre_scale):
                """AllReduce the [B, DN] f16 partial, squash into orep."""
                nc.sync.dma_start(cc_in[r][:], s_sb16[:])
                nc.gpsimd.collective_compute(
                    "AllReduce", OP.add, ins=[cc_in[r][:]],
                    outs=[cc_out[r][:]], replica_groups=[core_ids])
                s_all = tail.tile([B, DN], f16, tag="t_all")
                nc.sync.dma_start(s_all[:], cc_out[r][:])
                squash_to_outrep(s_all[:], orep, pre_scale)

            # u16 pool reserved before sweep 1 (validated geometry)
            u16p = _late.enter_context(tc.tile_pool(name="u16p", bufs=9))
            resident = {}

            # ---------------- sweep 1: u_hat + S0 ----------------
            with tc.tile_pool(name="xw", bufs=1) as xw:
                xbd_sb = xw.tile([128, NP, 128], f16)
                nc.sync.dma_start(xbd_sb[:], x_bd[:])
                subacc = [xw.tile([128, DN], f16, tag=f"sa{k}", name=f"sa{k}")
                          for k in range(8)]
                with tc.tile_pool(name="wp", bufs=4) as wp, \
                     tc.tile_pool(name="u16s1", bufs=4) as u16s1, \
                     tc.tile_pool(name="psum1", bufs=4, space="PSUM") as psum1:
                    for gp in range(NP):
                        wt = wp.tile([128, DN], f16)
                        nc.sync.dma_start(wt[:], w_t2[gp])
                        for gs in range(2):
                            g = 2 * gp + gs
                            u16 = u16s1.tile([128, DN], f16, tag="us1",
                                             name=f"us1_{g}")[:]
                            for h in range(2):
                                pu = psum1.tile([128, DN // 2], f32)
                                for ch in range(2):
                                    nc.tensor.matmul(
                                        pu[:, 512 * ch:512 * (ch + 1)],
                                        lhsT=xbd_sb[64 * gs:64 * (gs + 1),
                                                    gp, :],
                                        rhs=wt[64 * gs:64 * (gs + 1),
                                               1024 * h + 512 * ch:
                                               1024 * h + 512 * (ch + 1)],
                                        start=True, stop=True)
                                # drain: 3/4 of groups on ACT, 1/4 on DVE
                                if g % 4 == 3:
                                    nc.vector.tensor_copy(
                                        u16[:, 1024 * h:1024 * (h + 1)],
                                        pu[:])
                                else:
                                    nc.scalar.copy(
                                        u16[:, 1024 * h:1024 * (h + 1)],
                                        pu[:])
                            nc.sync.dma_start(u_store[g], u16)
                            # accumulate S0 on DVE in f16 sub-accumulators
                            sa = subacc[g // 8]
                            if g % 8 == 0:
                                nc.vector.tensor_copy(sa[:], u16)
                            else:
                                nc.vector.tensor_add(sa[:], sa[:], u16)

                # merge sub-accumulators (f16), fold j-slots via matmul
                for a, b_ in [(0, 1), (2, 3), (4, 5), (6, 7), (0, 2),
                              (4, 6), (0, 4)]:
                    nc.vector.tensor_add(subacc[a][:], subacc[a][:],
                                         subacc[b_][:])
                psacc = _late.enter_context(
                    tc.tile_pool(name="psacc", bufs=1, space="PSUM"))
                s0_ps = psacc.tile([B, DN], f32, tag="sacc")
                for ch in range(4):
                    nc.tensor.matmul(
                        s0_ps[:, 512 * ch:512 * (ch + 1)],
                        lhsT=ones_sb[:],
                        rhs=subacc[0][:, 512 * ch:512 * (ch + 1)],
                        start=True, stop=True)

                # S0 exchange + squash -> out_rep[0]
                s0_dr = tail.tile([B, DN], f16, tag="t_drain")
                nc.scalar.copy(s0_dr[:], s0_ps[:])
                exchange_and_squash(s0_dr[:], 0, out_rep[0], 1.0 / 64.0)

            # ---------------- sweeps 2 and 3: routing ----------------
            # b-state: [128, NP, 2, N] f32 mega-tile persistent across sweeps
            bs = bstate.tile([128, NP, 2, N], f32)
            # processing order: resident pairs first (both sweeps use the
            # same order; bs is indexed by position, u_store by gp)
            order = list(range(NP))
            with tc.tile_pool(name="big", bufs=2) as big, \
                 tc.tile_pool(name="tree", bufs=2) as tree, \
                 tc.tile_pool(name="soft", bufs=2) as soft:
                for it in range(2):
                    s_ps = psacc.tile([B, DN], f32, tag="sacc")
                    first_mm = True
                    for w in range(NP // WIN):
                        u16s = []
                        t4 = soft.tile([128, WIN, 2, 4, N], f16, tag="t4")
                        for pw in range(WIN):
                            gp = order[WIN * w + pw]
                            if it == 0 and gp in resident:
                                u16 = resident[gp]
                            else:
                                u16 = u16p.tile([128, 2, DN], f16,
                                                tag="u16",
                                                name=f"u{it}_{gp}")
                                nc.sync.dma_start(
                                    u16[:],
                                    u_store[2 * gp:2 * gp + 2]
                                    .transpose([1, 0, 2]))
                            u16s.append(u16)
                            u4 = u16[:].rearrange("p a (d n) -> p a d n",
                                                  n=N)
                            orep4 = (out_rep[it][:]
                                     .rearrange("p (d n) -> p d n", n=N)
                                     .unsqueeze(1)
                                     .broadcast_to([128, 2, D, N]))
                            tmp = big.tile([128, 2, D, N], f16, tag="tmp")
                            nc.vector.tensor_mul(tmp[:], u4, orep4)
                            # per-pair tree down to d=4, last level lands
                            # in the shared window tile t4
                            cur, d = tmp, D
                            while d > 8:
                                nxt = tree.tile([128, 2, d // 2, N], f16,
                                                tag=f"tr{d}")
                                nc.vector.tensor_add(
                                    nxt[:], cur[:, :, 0:d // 2, :],
                                    cur[:, :, d // 2:d, :])
                                cur, d = nxt, d // 2
                            nc.vector.tensor_add(
                                t4[:, pw], cur[:, :, 0:4, :],
                                cur[:, :, 4:8, :])

                        # batched tree tail + agreement for the window
                        bsw = bs[:, WIN * w:WIN * (w + 1), :, :]
                        t2 = soft.tile([128, WIN, 2, 2, N], f16, tag="t2")
                        nc.vector.tensor_add(t2[:], t4[:, :, :, 0:2, :],
                                             t4[:, :, :, 2:4, :])
                        if it == 0:
                            nc.vector.tensor_add(
                                bsw, t2[:, :, :, 0, :], t2[:, :, :, 1, :])
                        else:
                            a2b = soft.tile([128, WIN, 2, N], f32,
                                            tag="a2b")
                            nc.vector.tensor_add(
                                a2b[:], t2[:, :, :, 0, :],
                                t2[:, :, :, 1, :])
                            nc.vector.tensor_add(bsw, bsw, a2b[:])

                        # batched softmax for window w (WIN pairs)
                        m8 = soft.tile([128, WIN * 2], f32, tag="m8")
                        nc.vector.tensor_reduce(
                            out=m8[:].rearrange("p (a c) -> p a c", c=2)
                                     .unsqueeze(-1),
                            in_=bsw, axis=AX.X, op=OP.max)
                        nm8 = soft.tile([128, WIN * 2], f32, tag="nm8")
                        nc.vector.tensor_scalar_mul(nm8[:], m8[:],
                                                    -INV_LOG2)
                        e8 = soft.tile([128, WIN, 2, N], f16, tag="e8")
                        z8 = soft.tile([128, WIN * 2], f32, tag="z8")
                        for pw in range(WIN):
                            for gs in range(2):
                                k = 2 * pw + gs
                                nc.scalar.activation(
                                    e8[:, pw, gs, :],
                                    bs[:, WIN * w + pw, gs, :],
                                    AF.Exp, bias=nm8[:, k:k + 1],
                                    scale=INV_LOG2,
                                    accum_out=z8[:, k:k + 1])
                        rz8 = soft.tile([128, WIN * 2], f32, tag="rz8")
                        nc.vector.reciprocal(rz8[:], z8[:])
                        # 1/Z folded into the fold-matmul lhsT via ACT
                        cz = soft.tile([128, WIN, 2, 32], f16, tag="cz")
                        for pw in range(WIN):
                            for gs in range(2):
                                k = 2 * pw + gs
                                nc.scalar.activation(
                                    cz[:, pw, gs, :], ones_sb[:],
                                    AF.Copy, bias=0.0,
                                    scale=rz8[:, k:k + 1])

                        # weight-apply + fold per pair
                        for pw in range(WIN):
                            pos = WIN * w + pw
                            u16 = u16s[pw]
                            u4 = u16[:].rearrange("p a (d n) -> p a d n",
                                                  n=N)
                            erep = (e8[:, pw, :, :].unsqueeze(2)
                                    .broadcast_to([128, 2, D, N]))
                            sm = big.tile([128, 2, D, N], f16, tag="sm")
                            nc.vector.tensor_mul(sm[:], u4, erep)
                            smf = sm[:].rearrange("p a d n -> p a (d n)")
                            for gs in range(2):
                                for ch in range(4):
                                    nc.tensor.matmul(
                                        s_ps[:, 512 * ch:512 * (ch + 1)],
                                        lhsT=cz[:, pw, gs, :],
                                        rhs=smf[:, gs,
                                                512 * ch:512 * (ch + 1)],
                                        start=first_mm,
                                        stop=(pos == NP - 1 and gs == 1),
                                        skip_group_check=True)
                                first_mm = False

                    if it == 0:
                        s_sb = tail.tile([B, DN], f16, tag="t_drain")
                        nc.scalar.copy(s_sb[:], s_ps[:])
                        exchange_and_squash(s_sb[:], 1, out_rep[1], 1.0)
                    else:
                        s_f = tail.tile([B, DN], f32, tag="t_fin")
                        nc.scalar.copy(s_f[:], s_ps[:])
                        nc.sync.dma_start(s2_part[:], s_f[:])
            _late.close()

    nc.compile()
    return nc


_NC_CACHE = {}


def _get_nc():
    if "nc" not in _NC_CACHE:
        _NC_CACHE["nc"] = _build()
    return _NC_CACHE["nc"]


def _prep_core(x_c, w_c):
    """x_c [B, IL, K] f32, w_c [N, IL, D, K] f32 -> in_map dict."""
    wt = np.ascontiguousarray(w_c.transpose(1, 3, 2, 0))  # [IL, K, D, N]
    wt2 = wt.reshape(NP, 8, K, DN).reshape(NP, 128, DN).astype(np.float16)
    xt = x_c.transpose(1, 2, 0)  # [IL, K, B]
    x_bd = np.zeros((128, NP, 128), np.float16)
    for g in range(G):
        q, s = g // 2, g % 2
        for j in range(4):
            i = 4 * g + j
            x_bd[s * 64 + j * 16:s * 64 + j * 16 + K, q,
                 j * 32:j * 32 + 32] = xt[i].astype(np.float16)
    return {"w_t2": wt2, "x_bd": x_bd}


def _squash_np(v):
    sn = np.sum(v * v, axis=-1, keepdims=True)
    return np.sqrt(sn) / (1.0 + sn) * v


def _run(inputs, W, trace=False):
    _install_ntff_hook()
    nc = _get_nc()
    x = np.asarray(inputs, np.float32)
    Wf = np.asarray(W, np.float32)
    in_maps = []
    for c in range(NCORES):
        sl = slice(c * IL, (c + 1) * IL)
        in_maps.append(_prep_core(x[:, sl, :], Wf[:, sl, :, :]))
    res = run_bass_kernel_spmd(nc, in_maps, list(range(NCORES)), trace=trace)
    s2 = np.zeros((B, DN), np.float64)
    for c in range(NCORES):
        s2 += res.results[c]["s2_part"].astype(np.float64)
    s2 = s2.reshape(B, D, N).transpose(0, 2, 1).astype(np.float32)
    out = _squash_np(s2).astype(np.float32)
    return out, res


def kernel(inputs, W):
    out, _ = _run(inputs, W, trace=False)
    return out
